# revision 7
# baseline (speedup 1.0000x reference)
"""MoE router kernel (CityExpertRouter) for 8 Trainium2 NeuronCores.

reference:
    logits = einsum("bld,ed->ble", x[8,4096,2048]f32, gate_w[16,2048]f32)
    probs = softmax(logits); w, i = top_k(probs, 2); w /= w.sum(-1)
    returns (w [8,4096,2] f32, i [8,4096,2] i32)

Math simplification: softmax + top2 + renorm collapses to
    w1 = 1/(1+exp(l2-l1)), w2 = 1-w1   (l1, l2 = top-2 logits)
so only the top-2 logits (values + indices) are needed on-chip.

Strategy (DMA-bound problem; the cost floor is x bytes / DMA bandwidth):
  - Data parallel over batch: core i gets x[i] (4096 tokens).
  - 3-byte x encoding instead of 4: x = fp16(x) + 2^-11 * e3m4 residual.
    Host splits x into xhi = fp16(x) (2B) and xlo = e3m4((x-xhi)*2^11)
    (1B), cutting HBM traffic 25% below fp32 while keeping the logit
    quantization error ~2^-16 relative (top-2 index flips ~2/262144
    tokens, rel err ~2e-3, well under the 2e-2 gate).
  - Gate weight is consumed as three stationary operands:
      w16 = [fp16(w) | fp16((w-fp16(w))*2^11)]  (hi path, exact to 2^-23)
      w8  = e3m4(w*2^5)                          (lo path)
  - Device, per token-group (15 groups of 256, then 128/64/32/32 so the
    pipeline that trails the final DMA byte is as short as possible;
    multi-buffered so the x-load DMAs stream back-to-back at line rate):
      * hi chain: matmuls (lhsT=[whi|wlo] fp16 [128,32]) -> ps[0:32]
      * lo chain: matmuls (lhsT=w8 e3m4 [128,16])        -> ps[32:48]
      * copy psum -> SBUF, then one fp32 matmul per 128-token block with
        rhs = stacked scaled identity [48,16] does transpose AND the
        hi/lo fold in one shot:
        out[t,e] = lgT[e,t] + 2^-11 lgT[16+e,t] + 2^-16 lgT[32+e,t]
      * DVE max/max_index read the folded logits straight from PSUM
        (top-8 sorted) -> top-2 values+indices
      * ACT sigmoid -> weights; big groups use a DVE sub + 2 sigmoids,
        tail groups use bias-AP sigmoids (sigmoid(-1*l2 + l1)) straight
        from vals so the ACT work needs no extra DVE hop
      * sub-128-token tail groups are packed into the last 128-token
        staging block at partition offsets 0/64/96 (the allowed AP start
        partitions), so every engine op keeps in/out on the same lanes
  - Stores: bulk store of blocks 0..29 issued once those groups are
    done (lands on otherwise-idle queues); only the last two 128-token
    blocks sit on the tail (w via scalar HWDGE, i via gpsimd SWDGE so
    the two descriptor paths run in parallel)
  - Scheduling notes: stores+const loads ride the scalar-engine HWDGE
    queue so the SP queue is purely x-loads (no head-of-line blocking);
    only the top-2 indices are staged/stored (top-8 scratch stays in
    SBUF), keeping the output DMA small.
"""

import numpy as np
import ml_dtypes

import concourse.bass as bass
import concourse.tile as tile
from concourse import bacc, mybir
from concourse.bass import ts
from concourse.bass_utils import run_bass_kernel_spmd

F16 = np.float16
F8 = ml_dtypes.float8_e3m4

B, L, D, E = 8, 4096, 2048, 16
T = L              # tokens per core (shard over batch dim)
C = D // 128       # 16 contraction chunks
NB = T // 128      # 32 staging blocks of 128 tokens

# groups: big steady-state groups, then a shrinking tail. (t0, size) pairs;
# tail groups are ordered so matmul output base partitions stay in {0,32,64}
# while the LAST group is small (short post-DMA pipeline).
GROUPS = [(i * 256, 256) for i in range(15)] + [
    (3840, 128),  # block 30
    (4032, 64),   # block 31, partitions 64:128
    (3968, 32),   # block 31, partitions 0:32
    (4000, 32),   # block 31, partitions 32:64
]
assert sum(sz for _, sz in GROUPS) == T

# power-of-two scales for the 3-way split (all exact in fp)
S_XLO = 2.0 ** 11   # x residual pre-scale
S_WLO = 2.0 ** 11   # w fp16 residual pre-scale
S_W8 = 2.0 ** 5     # w e3m4 pre-scale

_CACHED_NC = None


def _build_nc():
    dt = mybir.dt
    nc = bacc.Bacc(
        "TRN2", target_bir_lowering=False, debug=False, num_devices=B
    )
    xhi_d = [
        nc.dram_tensor(f"xhi{g}", [128, C, sz], dt.float16, kind="ExternalInput")
        for g, (_, sz) in enumerate(GROUPS)
    ]
    xlo_d = [
        nc.dram_tensor(f"xlo{g}", [128, C, sz], dt.float8e3, kind="ExternalInput")
        for g, (_, sz) in enumerate(GROUPS)
    ]
    w16_d = nc.dram_tensor("w16", [128, C, 2 * E], dt.float16, kind="ExternalInput")
    w8_d = nc.dram_tensor("w8", [128, C, E], dt.float8e3, kind="ExternalInput")
    fold_d = nc.dram_tensor("fold", [3 * E, E], dt.float32, kind="ExternalInput")
    # device-native layout [p, b, k]; host un-permutes to [token, k]
    wout_d = nc.dram_tensor("w_out", [128, NB, 2], dt.float32, kind="ExternalOutput")
    iout_d = nc.dram_tensor("i_out", [128, NB, 2], dt.uint32, kind="ExternalOutput")

    with tile.TileContext(nc) as tc:
        with (
            tc.tile_pool(name="consts", bufs=1) as consts,
            tc.tile_pool(name="xin", bufs=3) as xin,
            tc.tile_pool(name="work", bufs=2) as work,
            tc.tile_pool(name="psum", bufs=2, space="PSUM") as psum_pool,
        ):
            fold_sb = consts.tile([3 * E, E], dt.float32)
            w16_sb = consts.tile([128, C, 2 * E], dt.float16)
            w8_sb = consts.tile([128, C, E], dt.float8e3)
            w_all = consts.tile([128, NB, 2], dt.float32)
            i_all = consts.tile([128, NB, 2], dt.uint32)

            for g, (t0, sz) in enumerate(GROUPS):
                xh = xin.tile([128, C, sz], dt.float16, name=f"xh_{sz}_{g % 3}")
                nc.sync.dma_start(xh[:], xhi_d[g][:])
                xl = xin.tile([128, C, sz], dt.float8e3, name=f"xl_{sz}_{g % 3}")
                nc.sync.dma_start(xl[:], xlo_d[g][:])
                if g == 0:
                    # consts go on the scalar HWDGE queue; SP queue stays
                    # pure x-loads
                    nc.scalar.dma_start(w16_sb[:], w16_d[:])
                    nc.scalar.dma_start(w8_sb[:], w8_d[:])
                    nc.scalar.dma_start(fold_sb[:], fold_d[:])

                # logitsT accumulation:
                #   rows [0:16]=whi@xhi, [16:32]=wlo@xhi, [32:48]=w8@xlo
                ps_full = psum_pool.tile([3 * E, 256], dt.float32, name="ps")
                ps = ps_full[:, :sz]
                for c in range(C):
                    nc.tensor.matmul(
                        ps[0 : 2 * E, :],
                        w16_sb[:, c, :],
                        xh[:, c, :],
                        start=(c == 0),
                        stop=(c == C - 1),
                    )
                for c in range(C):
                    nc.tensor.matmul(
                        ps[2 * E : 3 * E, :],
                        w8_sb[:, c, :],
                        xl[:, c, :],
                        start=(c == 0),
                        stop=(c == C - 1),
                    )

                lg32 = work.tile([3 * E, sz], dt.float32, name=f"lg{sz}")
                nc.vector.tensor_copy(lg32[:], ps[:])

                # transpose+fold:
                #   out[t,e] = lgT[e,t] + 2^-11 lgT[16+e,t] + 2^-16 lgT[32+e,t]
                # tokens land on partitions; sub-128 groups sit at the
                # partition offset of their slot in the staging block
                J = max(1, sz // 128)
                po = t0 % 128  # 0 for full blocks; 0/64/96 for tail groups
                b0 = t0 // 128
                pn = min(sz, 128)
                pt_full = psum_pool.tile([128, 2, E], dt.float32, name="pt")
                pt = pt_full[:, :J, :]
                for j in range(J):
                    nc.tensor.matmul(
                        pt[po : po + pn, j, :],
                        lg32[:, ts(j, 128)] if sz >= 128 else lg32[:, :],
                        fold_sb[:],
                        start=True,
                        stop=True,
                    )

                vals = work.tile([128, J, 8], dt.float32, name=f"vals{J}")
                idx8 = work.tile([128, J, 8], dt.uint32, name=f"idx{J}")
                for j in range(J):
                    # top-8 straight from PSUM
                    nc.vector.max(vals[po : po + pn, j, :], pt[po : po + pn, j, :])
                    nc.vector.max_index(
                        idx8[po : po + pn, j, :],
                        vals[po : po + pn, j, :],
                        pt[po : po + pn, j, :],
                    )
                # stage only the top-2 indices (uint32 -> int32 free on host)
                nc.vector.tensor_copy(
                    i_all[po : po + pn, b0 : b0 + J, :], idx8[po : po + pn, :, 0:2]
                )

                # w1 = sigmoid(l1-l2), w2 = sigmoid(l2-l1); renormalized top-2
                if sz >= 128:
                    dd = work.tile([128, J], dt.float32, name=f"dd{J}")
                    nc.vector.tensor_sub(dd[:], vals[:, :, 1], vals[:, :, 0])
                    nc.scalar.activation(
                        w_all[:, b0 : b0 + J, 0], dd[:],
                        mybir.ActivationFunctionType.Sigmoid, scale=-1.0,
                    )
                    nc.scalar.activation(
                        w_all[:, b0 : b0 + J, 1], dd[:],
                        mybir.ActivationFunctionType.Sigmoid,
                    )
                else:
                    # tail: skip the DVE sub; sigmoid(-1*l_other + l_this)
                    nc.scalar.activation(
                        w_all[po : po + pn, b0, 0:1],
                        vals[po : po + pn, 0, 1:2],
                        mybir.ActivationFunctionType.Sigmoid,
                        bias=vals[po : po + pn, 0, 0:1],
                        scale=-1.0,
                    )
                    nc.scalar.activation(
                        w_all[po : po + pn, b0, 1:2],
                        vals[po : po + pn, 0, 0:1],
                        mybir.ActivationFunctionType.Sigmoid,
                        bias=vals[po : po + pn, 0, 1:2],
                        scale=-1.0,
                    )
                if g == 14:
                    # bulk store of finished blocks 0..29; lands right after
                    # the last x loads on otherwise-idle queues
                    nc.gpsimd.dma_start(iout_d[:, :30], i_all[:, :30])
                    nc.scalar.dma_start(wout_d[:, :30], w_all[:, :30])

            # tail stores (blocks 30..31): SWDGE for indices so descriptor
            # generation runs in parallel with the HWDGE path
            nc.gpsimd.dma_start(iout_d[:, 30:], i_all[:, 30:])
            nc.scalar.dma_start(wout_d[:, 30:], w_all[:, 30:])

    nc.compile()
    return nc


def _permute(m):
    """[sz, D] -> [p=128, c, sz] device layout (d = c*128 + p)."""
    sz = m.shape[0]
    return np.ascontiguousarray(m.reshape(sz, C, 128).transpose(2, 1, 0))


def make_in_maps(x, gate_w):
    x = np.asarray(x, dtype=np.float32)
    gate_w = np.asarray(gate_w, dtype=np.float32)

    # weight prep: [e, d] -> [p, c, e] with d = c*128 + p
    def wtr(m):
        return m.T.reshape(C, 128, E).transpose(1, 0, 2)

    whi = gate_w.astype(F16)
    wlo = ((gate_w - whi.astype(np.float32)) * np.float32(S_WLO)).astype(F16)
    w16 = np.ascontiguousarray(np.concatenate([wtr(whi), wtr(wlo)], axis=2))
    w8 = np.ascontiguousarray(wtr((gate_w * np.float32(S_W8)).astype(F8)))

    fold = np.concatenate(
        [
            np.eye(E, dtype=np.float32),
            np.eye(E, dtype=np.float32) / np.float32(S_WLO),
            np.eye(E, dtype=np.float32) / np.float32(S_XLO * S_W8),
        ],
        axis=0,
    )

    in_maps = []
    for i in range(B):
        xi = x[i]
        xhi = xi.astype(F16)
        xlo = ((xi - xhi.astype(np.float32)) * np.float32(S_XLO)).astype(F8)
        m = {"w16": w16, "w8": w8, "fold": fold}
        for g, (t0, sz) in enumerate(GROUPS):
            m[f"xhi{g}"] = _permute(xhi[t0 : t0 + sz])
            m[f"xlo{g}"] = _permute(xlo[t0 : t0 + sz])
        in_maps.append(m)
    return in_maps


def kernel(x, gate_w):
    global _CACHED_NC
    if _CACHED_NC is None:
        _CACHED_NC = _build_nc()
    nc = _CACHED_NC

    in_maps = make_in_maps(x, gate_w)
    res = run_bass_kernel_spmd(nc, in_maps, list(range(B)))

    def unperm(a):  # [p, b, k] -> [t, k] with t = b*128 + p
        return a.transpose(1, 0, 2).reshape(T, -1)

    weights = np.stack([unperm(res.results[i]["w_out"]) for i in range(B)], axis=0)
    indices = np.stack(
        [unperm(res.results[i]["i_out"]) for i in range(B)], axis=0
    )
    return weights.astype(np.float32), indices.astype(np.int32)


# revision 8
# speedup vs baseline: 1.0102x; 1.0102x over previous
"""MoE router kernel (CityExpertRouter) for 8 Trainium2 NeuronCores.

reference:
    logits = einsum("bld,ed->ble", x[8,4096,2048]f32, gate_w[16,2048]f32)
    probs = softmax(logits); w, i = top_k(probs, 2); w /= w.sum(-1)
    returns (w [8,4096,2] f32, i [8,4096,2] i32)

Math simplification: softmax + top2 + renorm collapses to
    w1 = 1/(1+exp(l2-l1)), w2 = 1-w1   (l1, l2 = top-2 logits)
so only the top-2 logits (values + indices) are needed on-chip.

Strategy (DMA-bound problem; the cost floor is x bytes / DMA bandwidth):
  - Data parallel over batch: core i gets x[i] (4096 tokens).
  - 3-byte x encoding instead of 4: x = fp16(x) + 2^-11 * e3m4 residual.
    Host splits x into xhi = fp16(x) (2B) and xlo = e3m4((x-xhi)*2^11)
    (1B), cutting HBM traffic 25% below fp32 while keeping the logit
    quantization error ~2^-16 relative (top-2 index flips ~2/262144
    tokens, rel err ~2e-3, well under the 2e-2 gate).
  - Gate weight is consumed as three stationary operands:
      w16 = [fp16(w) | fp16((w-fp16(w))*2^11)]  (hi path, exact to 2^-23)
      w8  = e3m4(w*2^5)                          (lo path)
  - Device, per token-group (15 groups of 256, then 128/64/32/32 so the
    pipeline that trails the final DMA byte is as short as possible;
    multi-buffered so the x-load DMAs stream back-to-back at line rate):
      * hi chain: matmuls (lhsT=[whi|wlo] fp16 [128,32]) -> ps[0:32]
      * lo chain: matmuls (lhsT=w8 e3m4 [128,16])        -> ps[32:48]
      * copy psum -> SBUF, then one fp32 matmul per 128-token block with
        rhs = stacked scaled identity [48,16] does transpose AND the
        hi/lo fold in one shot:
        out[t,e] = lgT[e,t] + 2^-11 lgT[16+e,t] + 2^-16 lgT[32+e,t]
      * DVE max/max_index read the folded logits straight from PSUM
        (top-8 sorted) -> top-2 values+indices
      * ACT sigmoid -> weights; big groups use a DVE sub + 2 sigmoids,
        tail groups use bias-AP sigmoids (sigmoid(-1*l2 + l1)) straight
        from vals so the ACT work needs no extra DVE hop
      * sub-128-token tail groups are packed into the last 128-token
        staging block at partition offsets 0/64/96 (the allowed AP start
        partitions), so every engine op keeps in/out on the same lanes
  - Stores: bulk store of blocks 0..29 issued once those groups are
    done (lands on otherwise-idle queues); only the last two 128-token
    blocks sit on the tail (w via scalar HWDGE, i via gpsimd SWDGE so
    the two descriptor paths run in parallel)
  - Scheduling notes: stores+const loads ride the scalar-engine HWDGE
    queue so the SP queue is purely x-loads (no head-of-line blocking);
    only the top-2 indices are staged/stored (top-8 scratch stays in
    SBUF), keeping the output DMA small.
"""

import numpy as np
import ml_dtypes

import concourse.bass as bass
import concourse.tile as tile
from concourse import bacc, mybir
from concourse.bass import ts
from concourse.bass_utils import run_bass_kernel_spmd

F16 = np.float16
F8 = ml_dtypes.float8_e3m4

B, L, D, E = 8, 4096, 2048, 16
T = L              # tokens per core (shard over batch dim)
C = D // 128       # 16 contraction chunks
NB = T // 128      # 32 staging blocks of 128 tokens

# groups: big steady-state groups, then a shrinking tail. (t0, size) pairs;
# tail groups are ordered so matmul output base partitions stay in {0,32,64}
# while the LAST group is small (short post-DMA pipeline).
GROUPS = [(i * 256, 256) for i in range(15)] + [
    (3840, 128),  # block 30
    (4032, 64),   # block 31, partitions 64:128
    (3968, 32),   # block 31, partitions 0:32
    (4000, 32),   # block 31, partitions 32:64
]
assert sum(sz for _, sz in GROUPS) == T

# power-of-two scales for the 3-way split (all exact in fp)
S_XLO = 2.0 ** 11   # x residual pre-scale
S_WLO = 2.0 ** 11   # w fp16 residual pre-scale
S_W8 = 2.0 ** 5     # w e3m4 pre-scale

_CACHED_NC = None


def _build_nc():
    dt = mybir.dt
    nc = bacc.Bacc(
        "TRN2", target_bir_lowering=False, debug=False, num_devices=B
    )
    xhi_d = [
        nc.dram_tensor(f"xhi{g}", [128, C, sz], dt.float16, kind="ExternalInput")
        for g, (_, sz) in enumerate(GROUPS)
    ]
    xlo_d = [
        nc.dram_tensor(f"xlo{g}", [128, C, sz], dt.float8e3, kind="ExternalInput")
        for g, (_, sz) in enumerate(GROUPS)
    ]
    w16_d = nc.dram_tensor("w16", [128, C, 2 * E], dt.float16, kind="ExternalInput")
    w8_d = nc.dram_tensor("w8", [128, C, E], dt.float8e3, kind="ExternalInput")
    fold_d = nc.dram_tensor("fold", [3 * E, E], dt.float32, kind="ExternalInput")
    # device-native layout [p, b, k]; host un-permutes to [token, k]
    wout_d = nc.dram_tensor("w_out", [128, NB, 2], dt.float32, kind="ExternalOutput")
    iout_d = nc.dram_tensor("i_out", [128, NB, 2], dt.uint32, kind="ExternalOutput")

    with tile.TileContext(nc) as tc:
        with (
            tc.tile_pool(name="consts", bufs=1) as consts,
            tc.tile_pool(name="xin", bufs=3) as xin,
            tc.tile_pool(name="work", bufs=2) as work,
            tc.tile_pool(name="psum", bufs=2, space="PSUM") as psum_pool,
        ):
            fold_sb = consts.tile([3 * E, E], dt.float32)
            w16_sb = consts.tile([128, C, 2 * E], dt.float16)
            w8_sb = consts.tile([128, C, E], dt.float8e3)
            w_all = consts.tile([128, NB, 2], dt.float32)
            i_all = consts.tile([128, NB, 2], dt.uint32)

            def phase1(g, t0, sz):
                """x loads + the PSUM accumulation chains for group g."""
                xh = xin.tile([128, C, sz], dt.float16, name=f"xh_{sz}_{g % 3}")
                nc.sync.dma_start(xh[:], xhi_d[g][:])
                xl = xin.tile([128, C, sz], dt.float8e3, name=f"xl_{sz}_{g % 3}")
                nc.sync.dma_start(xl[:], xlo_d[g][:])
                if g == 0:
                    # consts go on the scalar HWDGE queue; SP queue stays
                    # pure x-loads
                    nc.scalar.dma_start(w16_sb[:], w16_d[:])
                    nc.scalar.dma_start(w8_sb[:], w8_d[:])
                    nc.scalar.dma_start(fold_sb[:], fold_d[:])

                # logitsT accumulation:
                #   rows [0:16]=whi@xhi, [16:32]=wlo@xhi, [32:48]=w8@xlo
                ps_full = psum_pool.tile([3 * E, 256], dt.float32, name="ps")
                ps = ps_full[:, :sz]
                for c in range(C):
                    nc.tensor.matmul(
                        ps[0 : 2 * E, :],
                        w16_sb[:, c, :],
                        xh[:, c, :],
                        start=(c == 0),
                        stop=(c == C - 1),
                    )
                for c in range(C):
                    nc.tensor.matmul(
                        ps[2 * E : 3 * E, :],
                        w8_sb[:, c, :],
                        xl[:, c, :],
                        start=(c == 0),
                        stop=(c == C - 1),
                    )
                return ps

            def phase2(g, t0, sz, ps):
                """psum -> logits fold -> top-2 -> weights for group g."""
                lg32 = work.tile([3 * E, sz], dt.float32, name=f"lg{sz}")
                nc.vector.tensor_copy(lg32[:], ps[:])

                # transpose+fold:
                #   out[t,e] = lgT[e,t] + 2^-11 lgT[16+e,t] + 2^-16 lgT[32+e,t]
                # tokens land on partitions; sub-128 groups sit at the
                # partition offset of their slot in the staging block
                J = max(1, sz // 128)
                po = t0 % 128  # 0 for full blocks; 0/32/64 for tail groups
                b0 = t0 // 128
                pn = min(sz, 128)
                pt_full = psum_pool.tile([128, 2, E], dt.float32, name="pt")
                pt = pt_full[:, :J, :]
                for j in range(J):
                    nc.tensor.matmul(
                        pt[po : po + pn, j, :],
                        lg32[:, ts(j, 128)] if sz >= 128 else lg32[:, :],
                        fold_sb[:],
                        start=True,
                        stop=True,
                    )

                vals = work.tile([128, J, 8], dt.float32, name=f"vals{J}")
                idx8 = work.tile([128, J, 8], dt.uint32, name=f"idx{J}")
                for j in range(J):
                    # top-8 straight from PSUM
                    nc.vector.max(vals[po : po + pn, j, :], pt[po : po + pn, j, :])
                    nc.vector.max_index(
                        idx8[po : po + pn, j, :],
                        vals[po : po + pn, j, :],
                        pt[po : po + pn, j, :],
                    )
                # stage only the top-2 indices (uint32 -> int32 free on host)
                nc.vector.tensor_copy(
                    i_all[po : po + pn, b0 : b0 + J, :], idx8[po : po + pn, :, 0:2]
                )

                # w1 = sigmoid(l1-l2), w2 = sigmoid(l2-l1); renormalized top-2
                if sz >= 128:
                    dd = work.tile([128, J], dt.float32, name=f"dd{J}")
                    nc.vector.tensor_sub(dd[:], vals[:, :, 1], vals[:, :, 0])
                    nc.scalar.activation(
                        w_all[:, b0 : b0 + J, 0], dd[:],
                        mybir.ActivationFunctionType.Sigmoid, scale=-1.0,
                    )
                    nc.scalar.activation(
                        w_all[:, b0 : b0 + J, 1], dd[:],
                        mybir.ActivationFunctionType.Sigmoid,
                    )
                else:
                    # tail: skip the DVE sub; sigmoid(-1*l_other + l_this)
                    nc.scalar.activation(
                        w_all[po : po + pn, b0, 0:1],
                        vals[po : po + pn, 0, 1:2],
                        mybir.ActivationFunctionType.Sigmoid,
                        bias=vals[po : po + pn, 0, 0:1],
                        scale=-1.0,
                    )
                    nc.scalar.activation(
                        w_all[po : po + pn, b0, 1:2],
                        vals[po : po + pn, 0, 0:1],
                        mybir.ActivationFunctionType.Sigmoid,
                        bias=vals[po : po + pn, 0, 1:2],
                        scale=-1.0,
                    )

            # software pipeline: emit fold(g-1) AFTER chains(g) so the PE
            # never waits on the DVE psum copy (it hides under the next
            # group's accumulation chain)
            pending = None
            for g, (t0, sz) in enumerate(GROUPS):
                ps = phase1(g, t0, sz)
                if pending is not None:
                    phase2(*pending)
                pending = (g, t0, sz, ps)
            phase2(*pending)

            # single final stores: w rides the scalar queue right behind its
            # producer sigmoids; i takes the idle SP queue in parallel
            nc.sync.dma_start(iout_d[:], i_all[:])
            nc.scalar.dma_start(wout_d[:], w_all[:])

    nc.compile()
    return nc


def _permute(m):
    """[sz, D] -> [p=128, c, sz] device layout (d = c*128 + p)."""
    sz = m.shape[0]
    return np.ascontiguousarray(m.reshape(sz, C, 128).transpose(2, 1, 0))


def make_in_maps(x, gate_w):
    x = np.asarray(x, dtype=np.float32)
    gate_w = np.asarray(gate_w, dtype=np.float32)

    # weight prep: [e, d] -> [p, c, e] with d = c*128 + p
    def wtr(m):
        return m.T.reshape(C, 128, E).transpose(1, 0, 2)

    whi = gate_w.astype(F16)
    wlo = ((gate_w - whi.astype(np.float32)) * np.float32(S_WLO)).astype(F16)
    w16 = np.ascontiguousarray(np.concatenate([wtr(whi), wtr(wlo)], axis=2))
    w8 = np.ascontiguousarray(wtr((gate_w * np.float32(S_W8)).astype(F8)))

    fold = np.concatenate(
        [
            np.eye(E, dtype=np.float32),
            np.eye(E, dtype=np.float32) / np.float32(S_WLO),
            np.eye(E, dtype=np.float32) / np.float32(S_XLO * S_W8),
        ],
        axis=0,
    )

    in_maps = []
    for i in range(B):
        xi = x[i]
        xhi = xi.astype(F16)
        xlo = ((xi - xhi.astype(np.float32)) * np.float32(S_XLO)).astype(F8)
        m = {"w16": w16, "w8": w8, "fold": fold}
        for g, (t0, sz) in enumerate(GROUPS):
            m[f"xhi{g}"] = _permute(xhi[t0 : t0 + sz])
            m[f"xlo{g}"] = _permute(xlo[t0 : t0 + sz])
        in_maps.append(m)
    return in_maps


def kernel(x, gate_w):
    global _CACHED_NC
    if _CACHED_NC is None:
        _CACHED_NC = _build_nc()
    nc = _CACHED_NC

    in_maps = make_in_maps(x, gate_w)
    res = run_bass_kernel_spmd(nc, in_maps, list(range(B)))

    def unperm(a):  # [p, b, k] -> [t, k] with t = b*128 + p
        return a.transpose(1, 0, 2).reshape(T, -1)

    weights = np.stack([unperm(res.results[i]["w_out"]) for i in range(B)], axis=0)
    indices = np.stack(
        [unperm(res.results[i]["i_out"]) for i in range(B)], axis=0
    )
    return weights.astype(np.float32), indices.astype(np.int32)


# revision 11
# speedup vs baseline: 1.0297x; 1.0193x over previous
"""MoE router kernel (CityExpertRouter) for 8 Trainium2 NeuronCores.

reference:
    logits = einsum("bld,ed->ble", x[8,4096,2048]f32, gate_w[16,2048]f32)
    probs = softmax(logits); w, i = top_k(probs, 2); w /= w.sum(-1)
    returns (w [8,4096,2] f32, i [8,4096,2] i32)

Math simplification: softmax + top2 + renorm collapses to
    w1 = 1/(1+exp(l2-l1)), w2 = 1-w1   (l1, l2 = top-2 logits)
so only the top-2 logits (values + indices) are needed on-chip.

Strategy (DMA-bound problem; the cost floor is x bytes / DMA bandwidth):
  - Data parallel over batch: core i gets x[i] (4096 tokens).
  - 3-byte x encoding instead of 4: x = fp16(x) + 2^-11 * e3m4 residual.
    Host splits x into xhi = fp16(x) (2B) and xlo = e3m4((x-xhi)*2^11)
    (1B), cutting HBM traffic 25% below fp32 while keeping the logit
    quantization error ~2^-16 relative (top-2 index flips ~2/262144
    tokens, rel err ~2e-3, well under the 2e-2 gate).
  - Gate weight is consumed as two small moving operands:
      w16 = [fp16(w) | fp16((w-fp16(w))*2^11)]  (hi path, exact to 2^-23)
      w8  = e3m4(w*2^5)                          (lo path)
  - Token-major matmuls: the x chunk [128d, tokens] is the STATIONARY
    operand and the tiny gate weight [128d, 32|16] is the MOVING one, so
    each accumulation step costs only 32 (hi) / 16 (lo) PE cycles and
    the logits land directly as [tokens(partitions), expert-slots] in
    PSUM - no transpose/fold pass and no PSUM->SBUF logits copy at all.
    Per 128-token block: ps[:, 0:32] += xhi_c^T w16_c over 16 chunks,
    ps[:, 32:48] += xlo_c^T w8_c.
  - Per-block epilogue:
      * DVE scalar_tensor_tensor x2 straight from PSUM:
        l = (ps[:,32:48]*2^-16 + ps[:,0:16]) + ps[:,16:32]
      * DVE max/max_index (top-8 sorted) -> top-2 values+indices
      * ACT bias-AP sigmoids: w1 = sigmoid(-1*l2 + l1), w2 = sigmoid(-1*
        l1 + l2) straight from vals (no DVE sub needed)
  - Group sizes 15x256 then 128/64/32/32 (tail groups packed into the
    last staging block at partition offsets 64/0/32) so the pipeline
    trailing the final DMA byte is as short as possible.
  - Single final stores: w rides the scalar queue right behind its
    producer sigmoids, i takes the otherwise-idle SP queue in parallel;
    both transfer AFTER the x stream so they never contend with it.
  - Scheduling notes: const loads ride the scalar HWDGE queue so the SP
    queue is purely x-loads; only the top-2 indices are staged/stored.
"""

import numpy as np
import ml_dtypes

import concourse.bass as bass
import concourse.tile as tile
from concourse import bacc, mybir
from concourse.bass import ts
from concourse.bass_utils import run_bass_kernel_spmd

F16 = np.float16
F8 = ml_dtypes.float8_e3m4

B, L, D, E = 8, 4096, 2048, 16
T = L              # tokens per core (shard over batch dim)
C = D // 128       # 16 contraction chunks
NB = T // 128      # 32 staging blocks of 128 tokens

# groups: big steady-state groups, then a shrinking tail. (t0, size) pairs;
# tail groups are ordered so PSUM base partitions stay in {0,32,64} while
# the LAST group is small (short post-DMA pipeline).
GROUPS = [(i * 256, 256) for i in range(15)] + [
    (3840, 128),  # block 30
    (4032, 64),   # block 31, partitions 64:128
    (3968, 32),   # block 31, partitions 0:32
    (4000, 32),   # block 31, partitions 32:64
]
assert sum(sz for _, sz in GROUPS) == T

# power-of-two scales for the 3-way split (all exact in fp)
S_XLO = 2.0 ** 11   # x residual pre-scale
S_WLO = 2.0 ** 11   # w fp16 residual pre-scale
S_W8 = 2.0 ** 5     # w e3m4 pre-scale
S_LO = 1.0 / (S_XLO * S_W8)  # lo-psum fold scale 2^-16

_CACHED_NC = None


def _build_nc():
    dt = mybir.dt
    nc = bacc.Bacc(
        "TRN2", target_bir_lowering=False, debug=False, num_devices=B
    )
    xhi_d = [
        nc.dram_tensor(f"xhi{g}", [128, C, sz], dt.float16, kind="ExternalInput")
        for g, (_, sz) in enumerate(GROUPS)
    ]
    xlo_d = [
        nc.dram_tensor(f"xlo{g}", [128, C, sz], dt.float8e3, kind="ExternalInput")
        for g, (_, sz) in enumerate(GROUPS)
    ]
    w16_d = nc.dram_tensor("w16", [128, C, 2 * E], dt.float16, kind="ExternalInput")
    w8_d = nc.dram_tensor("w8", [128, C, E], dt.float8e3, kind="ExternalInput")
    # device-native layout [p, b, k]; host un-permutes to [token, k]
    wout_d = nc.dram_tensor("w_out", [128, NB, 2], dt.float32, kind="ExternalOutput")
    iout_d = nc.dram_tensor("i_out", [128, NB, 2], dt.uint32, kind="ExternalOutput")

    with tile.TileContext(nc) as tc:
        with (
            tc.tile_pool(name="consts", bufs=1) as consts,
            tc.tile_pool(name="xin", bufs=3) as xin,
            tc.tile_pool(name="work", bufs=4) as work,
            tc.tile_pool(name="psum", bufs=4, space="PSUM") as psum_pool,
        ):
            w16_sb = consts.tile([128, C, 2 * E], dt.float16)
            w8_sb = consts.tile([128, C, E], dt.float8e3)
            w_all = consts.tile([128, NB, 2], dt.float32)
            i_all = consts.tile([128, NB, 2], dt.uint32)

            for g, (t0, sz) in enumerate(GROUPS):
                xh = xin.tile([128, C, sz], dt.float16, name=f"xh_{sz}_{g % 3}")
                nc.sync.dma_start(xh[:], xhi_d[g][:])
                xl = xin.tile([128, C, sz], dt.float8e3, name=f"xl_{sz}_{g % 3}")
                nc.sync.dma_start(xl[:], xlo_d[g][:])
                if g == 0:
                    # consts go on the scalar HWDGE queue; SP queue stays
                    # pure x-loads
                    nc.scalar.dma_start(w16_sb[:], w16_d[:])
                    nc.scalar.dma_start(w8_sb[:], w8_d[:])

                nblk = max(1, sz // 128)
                po = t0 % 128  # 0 for full blocks; 0/32/64 for tail groups
                b0 = t0 // 128
                pn = min(sz, 128)

                # token-major accumulation, x stationary / w moving:
                #   ps[:, 0:16]=xhi@whi, [16:32]=xhi@wlo, [32:48]=xlo@w8
                pss = [
                    psum_pool.tile([128, 3 * E], dt.float32, name="ps")
                    for _ in range(nblk)
                ]
                # hi chains first (xhi arrives before xlo)
                for b, ps in enumerate(pss):
                    xs = xh[:, :, ts(b, 128)] if sz >= 128 else xh[:, :, :]
                    for c in range(C):
                        nc.tensor.matmul(
                            ps[po : po + pn, 0 : 2 * E],
                            xs[:, c, :],
                            w16_sb[:, c, :],
                            start=(c == 0),
                            stop=(c == C - 1),
                        )
                for b, ps in enumerate(pss):
                    xs = xl[:, :, ts(b, 128)] if sz >= 128 else xl[:, :, :]
                    for c in range(C):
                        nc.tensor.matmul(
                            ps[po : po + pn, 2 * E : 3 * E],
                            xs[:, c, :],
                            w8_sb[:, c, :],
                            start=(c == 0),
                            stop=(c == C - 1),
                        )

                for b, ps in enumerate(pss):
                    blk = b0 + b
                    # fold the three psum slots into fp32 logits on DVE.
                    # HW constraint: each op may read only ONE input from
                    # PSUM, so descale lo to SBUF first, then two adds.
                    t = work.tile([128, E], dt.float32, name="t")
                    nc.vector.tensor_scalar_mul(
                        t[po : po + pn, :], ps[po : po + pn, 2 * E : 3 * E], S_LO
                    )
                    s1 = work.tile([128, E], dt.float32, name="s1")
                    nc.vector.scalar_tensor_tensor(
                        s1[po : po + pn, :],
                        ps[po : po + pn, 0:E],
                        1.0,
                        t[po : po + pn, :],
                        op0=mybir.AluOpType.mult,
                        op1=mybir.AluOpType.add,
                    )
                    lg = work.tile([128, E], dt.float32, name="lg")
                    nc.vector.scalar_tensor_tensor(
                        lg[po : po + pn, :],
                        ps[po : po + pn, E : 2 * E],
                        1.0 / S_WLO,
                        s1[po : po + pn, :],
                        op0=mybir.AluOpType.mult,
                        op1=mybir.AluOpType.add,
                    )

                    vals = work.tile([128, 8], dt.float32, name="vals")
                    idx8 = work.tile([128, 8], dt.uint32, name="idx8")
                    nc.vector.max(vals[po : po + pn, :], lg[po : po + pn, :])
                    nc.vector.max_index(
                        idx8[po : po + pn, :],
                        vals[po : po + pn, :],
                        lg[po : po + pn, :],
                    )
                    # stage only the top-2 indices (uint32 -> int32 on host)
                    nc.vector.tensor_copy(
                        i_all[po : po + pn, blk, :], idx8[po : po + pn, 0:2]
                    )
                    # w1 = sigmoid(l1-l2), w2 = sigmoid(l2-l1): bias-AP form
                    nc.scalar.activation(
                        w_all[po : po + pn, blk, 0:1],
                        vals[po : po + pn, 1:2],
                        mybir.ActivationFunctionType.Sigmoid,
                        bias=vals[po : po + pn, 0:1],
                        scale=-1.0,
                    )
                    nc.scalar.activation(
                        w_all[po : po + pn, blk, 1:2],
                        vals[po : po + pn, 0:1],
                        mybir.ActivationFunctionType.Sigmoid,
                        bias=vals[po : po + pn, 1:2],
                        scale=-1.0,
                    )

            # single final stores: w rides the scalar queue right behind its
            # producer sigmoids; i takes the idle SP queue in parallel
            nc.sync.dma_start(iout_d[:], i_all[:])
            nc.scalar.dma_start(wout_d[:], w_all[:])

    nc.compile()
    return nc


def _permute(m):
    """[sz, D] -> [p=128, c, sz] device layout (d = c*128 + p)."""
    sz = m.shape[0]
    return np.ascontiguousarray(m.reshape(sz, C, 128).transpose(2, 1, 0))


def make_in_maps(x, gate_w):
    x = np.asarray(x, dtype=np.float32)
    gate_w = np.asarray(gate_w, dtype=np.float32)

    # weight prep: [e, d] -> [p, c, e] with d = c*128 + p
    def wtr(m):
        return m.T.reshape(C, 128, E).transpose(1, 0, 2)

    whi = gate_w.astype(F16)
    wlo = ((gate_w - whi.astype(np.float32)) * np.float32(S_WLO)).astype(F16)
    w16 = np.ascontiguousarray(np.concatenate([wtr(whi), wtr(wlo)], axis=2))
    w8 = np.ascontiguousarray(wtr((gate_w * np.float32(S_W8)).astype(F8)))

    in_maps = []
    for i in range(B):
        xi = x[i]
        xhi = xi.astype(F16)
        xlo = ((xi - xhi.astype(np.float32)) * np.float32(S_XLO)).astype(F8)
        m = {"w16": w16, "w8": w8}
        for g, (t0, sz) in enumerate(GROUPS):
            m[f"xhi{g}"] = _permute(xhi[t0 : t0 + sz])
            m[f"xlo{g}"] = _permute(xlo[t0 : t0 + sz])
        in_maps.append(m)
    return in_maps


def kernel(x, gate_w):
    global _CACHED_NC
    if _CACHED_NC is None:
        _CACHED_NC = _build_nc()
    nc = _CACHED_NC

    in_maps = make_in_maps(x, gate_w)
    res = run_bass_kernel_spmd(nc, in_maps, list(range(B)))

    def unperm(a):  # [p, b, k] -> [t, k] with t = b*128 + p
        return a.transpose(1, 0, 2).reshape(T, -1)

    weights = np.stack([unperm(res.results[i]["w_out"]) for i in range(B)], axis=0)
    indices = np.stack(
        [unperm(res.results[i]["i_out"]) for i in range(B)], axis=0
    )
    return weights.astype(np.float32), indices.astype(np.int32)


# revision 13
# speedup vs baseline: 1.0601x; 1.0295x over previous
"""MoE router kernel (CityExpertRouter) for 8 Trainium2 NeuronCores.

reference:
    logits = einsum("bld,ed->ble", x[8,4096,2048]f32, gate_w[16,2048]f32)
    probs = softmax(logits); w, i = top_k(probs, 2); w /= w.sum(-1)
    returns (w [8,4096,2] f32, i [8,4096,2] i32)

Math simplification: softmax + top2 + renorm collapses to
    w1 = 1/(1+exp(l2-l1)), w2 = 1-w1   (l1, l2 = top-2 logits)
so only the top-2 logits (values + indices) are needed on-chip.

Strategy (DMA-bound problem; the cost floor is x bytes / DMA bandwidth):
  - Data parallel over batch: core i gets x[i] (4096 tokens).
  - 3-byte x encoding instead of 4: x = fp16(x) + 2^-11 * e3m4 residual.
    Host splits x into xhi = fp16(x) (2B) and xlo = e3m4((x-xhi)*2^11)
    (1B), cutting HBM traffic 25% below fp32 while keeping the logit
    quantization error ~2^-16 relative (top-2 index flips ~2/262144
    tokens, rel err ~2e-3, well under the 2e-2 gate).
  - Gate weight is consumed as two small moving operands:
      w16 = [fp16(w) | fp16((w-fp16(w))*2^11)]  (hi path, exact to 2^-23)
      w8  = e3m4(w*2^5)                          (lo path)
  - Token-major matmuls: the x chunk [128d, tokens] is the STATIONARY
    operand and the tiny gate weight [128d, 32|16] is the MOVING one, so
    each accumulation step costs only 32 (hi) / 16 (lo) PE cycles and
    the logits land directly as [tokens(partitions), expert-slots] in
    PSUM - no transpose/fold pass and no PSUM->SBUF logits copy at all.
    Per 128-token block: ps[:, 0:32] += xhi_c^T w16_c over 16 chunks,
    ps[:, 32:48] += xlo_c^T w8_c.
  - Per-block epilogue:
      * DVE scalar_tensor_tensor x2 straight from PSUM:
        l = (ps[:,32:48]*2^-16 + ps[:,0:16]) + ps[:,16:32]
      * DVE max/max_index (top-8 sorted) -> top-2 values+indices
      * ACT bias-AP sigmoids: w1 = sigmoid(-1*l2 + l1), w2 = sigmoid(-1*
        l1 + l2) straight from vals (no DVE sub needed)
  - Group sizes 15x256 then 128/64/32/32 (tail groups packed into the
    last staging block at partition offsets 64/0/32) so the pipeline
    trailing the final DMA byte is as short as possible.
  - Single final stores: w rides the scalar queue right behind its
    producer sigmoids, i takes the otherwise-idle SP queue in parallel;
    both transfer AFTER the x stream so they never contend with it.
  - Scheduling notes: const loads ride the scalar HWDGE queue so the SP
    queue is purely x-loads; only the top-2 indices are staged/stored.
"""

import numpy as np
import ml_dtypes

import concourse.bass as bass
import concourse.tile as tile
from concourse import bacc, mybir
from concourse.bass import ts
from concourse.bass_utils import run_bass_kernel_spmd

F16 = np.float16
F8 = ml_dtypes.float8_e3m4

B, L, D, E = 8, 4096, 2048, 16
T = L              # tokens per core (shard over batch dim)
C = D // 128       # 16 contraction chunks
NB = T // 128      # 32 staging blocks of 128 tokens

# groups: big steady-state groups, then a shrinking tail. (t0, size) pairs;
# tail groups are ordered so PSUM base partitions stay in {0,32,64} while
# the LAST group is small (short post-DMA pipeline).
GROUPS = [(i * 256, 256) for i in range(15)] + [
    (3840, 128),  # block 30
    (3968, 128),  # block 31 (the short tail group)
]
assert sum(sz for _, sz in GROUPS) == T

# power-of-two scales for the 3-way split (all exact in fp)
S_XLO = 2.0 ** 11   # x residual pre-scale
S_WLO = 2.0 ** 11   # w fp16 residual pre-scale
S_W8 = 2.0 ** 5     # w e3m4 pre-scale
S_LO = 1.0 / (S_XLO * S_W8)  # lo-psum fold scale 2^-16

_CACHED_NC = None


def _build_nc():
    dt = mybir.dt
    nc = bacc.Bacc(
        "TRN2", target_bir_lowering=False, debug=False, num_devices=B
    )
    xhi_d = [
        nc.dram_tensor(f"xhi{g}", [128, C, sz], dt.float16, kind="ExternalInput")
        for g, (_, sz) in enumerate(GROUPS)
    ]
    xlo_d = [
        nc.dram_tensor(f"xlo{g}", [128, C, sz], dt.float8e3, kind="ExternalInput")
        for g, (_, sz) in enumerate(GROUPS)
    ]
    w16_d = nc.dram_tensor("w16", [128, C, 2 * E], dt.float16, kind="ExternalInput")
    w8_d = nc.dram_tensor("w8", [128, C, E], dt.float8e3, kind="ExternalInput")
    # device-native layout [p, b, k]; host un-permutes to [token, k]
    wout_d = nc.dram_tensor("w_out", [128, NB, 2], dt.float32, kind="ExternalOutput")
    iout_d = nc.dram_tensor("i_out", [128, NB, 2], dt.uint32, kind="ExternalOutput")

    with tile.TileContext(nc) as tc:
        with (
            tc.tile_pool(name="consts", bufs=1) as consts,
            tc.tile_pool(name="xin", bufs=3) as xin,
            tc.tile_pool(name="work", bufs=4) as work,
            tc.tile_pool(name="psum", bufs=4, space="PSUM") as psum_pool,
        ):
            w16_sb = consts.tile([128, C, 2 * E], dt.float16)
            w8_sb = consts.tile([128, C, E], dt.float8e3)
            w_all = consts.tile([128, NB, 2], dt.float32)
            i_all = consts.tile([128, NB, 2], dt.uint32)

            for g, (t0, sz) in enumerate(GROUPS):
                xh = xin.tile([128, C, sz], dt.float16, name=f"xh_{sz}_{g % 3}")
                nc.sync.dma_start(xh[:], xhi_d[g][:])
                xl = xin.tile([128, C, sz], dt.float8e3, name=f"xl_{sz}_{g % 3}")
                nc.sync.dma_start(xl[:], xlo_d[g][:])
                if g == 0:
                    # consts go on the scalar HWDGE queue; SP queue stays
                    # pure x-loads
                    nc.scalar.dma_start(w16_sb[:], w16_d[:])
                    nc.scalar.dma_start(w8_sb[:], w8_d[:])

                nblk = max(1, sz // 128)
                po = t0 % 128  # 0 for full blocks; 0/32/64 for tail groups
                b0 = t0 // 128
                pn = min(sz, 128)

                # token-major accumulation, x stationary / w moving:
                #   ps[:, 0:16]=xhi@whi, [16:32]=xhi@wlo, [32:48]=xlo@w8
                pss = [
                    psum_pool.tile([128, 3 * E], dt.float32, name="ps")
                    for _ in range(nblk)
                ]
                # hi chains first (xhi arrives before xlo)
                for b, ps in enumerate(pss):
                    xs = xh[:, :, ts(b, 128)] if sz >= 128 else xh[:, :, :]
                    for c in range(C):
                        nc.tensor.matmul(
                            ps[po : po + pn, 0 : 2 * E],
                            xs[:, c, :],
                            w16_sb[:, c, :],
                            start=(c == 0),
                            stop=(c == C - 1),
                        )
                # pre-combine the hi psum slots on DVE while the lo bytes
                # are still in flight: hsum = whi-part + 2^-11 * wlo-part.
                # (HW allows only ONE PSUM input per DVE op, so two steps.)
                hsums = []
                for b, ps in enumerate(pss):
                    h1 = work.tile([128, E], dt.float32, name="h1")
                    nc.vector.tensor_scalar_mul(
                        h1[po : po + pn, :],
                        ps[po : po + pn, E : 2 * E],
                        1.0 / S_WLO,
                    )
                    hsum = work.tile([128, E], dt.float32, name="hsum")
                    nc.vector.scalar_tensor_tensor(
                        hsum[po : po + pn, :],
                        ps[po : po + pn, 0:E],
                        1.0,
                        h1[po : po + pn, :],
                        op0=mybir.AluOpType.mult,
                        op1=mybir.AluOpType.add,
                    )
                    hsums.append(hsum)

                for b, ps in enumerate(pss):
                    xs = xl[:, :, ts(b, 128)] if sz >= 128 else xl[:, :, :]
                    for c in range(C):
                        nc.tensor.matmul(
                            ps[po : po + pn, 2 * E : 3 * E],
                            xs[:, c, :],
                            w8_sb[:, c, :],
                            start=(c == 0),
                            stop=(c == C - 1),
                        )

                for b, ps in enumerate(pss):
                    blk = b0 + b
                    # single post-lo DVE op: l = 2^-16 * lo-part + hsum
                    lg = work.tile([128, E], dt.float32, name="lg")
                    nc.vector.scalar_tensor_tensor(
                        lg[po : po + pn, :],
                        ps[po : po + pn, 2 * E : 3 * E],
                        S_LO,
                        hsums[b][po : po + pn, :],
                        op0=mybir.AluOpType.mult,
                        op1=mybir.AluOpType.add,
                    )

                    vals = work.tile([128, 8], dt.float32, name="vals")
                    idx8 = work.tile([128, 8], dt.uint32, name="idx8")
                    nc.vector.max(vals[po : po + pn, :], lg[po : po + pn, :])
                    nc.vector.max_index(
                        idx8[po : po + pn, :],
                        vals[po : po + pn, :],
                        lg[po : po + pn, :],
                    )
                    if g < len(GROUPS) - 1:
                        # stage the top-2 indices (uint32 -> int32 on host)
                        nc.vector.tensor_copy(
                            i_all[po : po + pn, blk, :], idx8[po : po + pn, 0:2]
                        )
                    else:
                        # last block: skip staging; the tail i-store reads
                        # straight from the top-8 scratch
                        idx_last = idx8
                    # w1 = sigmoid(l1-l2), w2 = sigmoid(l2-l1): bias-AP form
                    nc.scalar.activation(
                        w_all[po : po + pn, blk, 0:1],
                        vals[po : po + pn, 1:2],
                        mybir.ActivationFunctionType.Sigmoid,
                        bias=vals[po : po + pn, 0:1],
                        scale=-1.0,
                    )
                    nc.scalar.activation(
                        w_all[po : po + pn, blk, 1:2],
                        vals[po : po + pn, 0:1],
                        mybir.ActivationFunctionType.Sigmoid,
                        bias=vals[po : po + pn, 1:2],
                        scale=-1.0,
                    )

            # final stores, split so the pieces gated by the last block are
            # tiny: w rides the scalar queue right behind its producer
            # sigmoids; i takes the otherwise-idle SP queue in parallel
            nc.sync.dma_start(iout_d[:, : NB - 1], i_all[:, : NB - 1])
            nc.scalar.dma_start(wout_d[:, : NB - 1], w_all[:, : NB - 1])
            nc.sync.dma_start(iout_d[:, NB - 1 :], idx_last[:, 0:2])
            nc.scalar.dma_start(wout_d[:, NB - 1 :], w_all[:, NB - 1 :])

    nc.compile()
    return nc


def _permute(m):
    """[sz, D] -> [p=128, c, sz] device layout (d = c*128 + p)."""
    sz = m.shape[0]
    return np.ascontiguousarray(m.reshape(sz, C, 128).transpose(2, 1, 0))


def make_in_maps(x, gate_w):
    x = np.asarray(x, dtype=np.float32)
    gate_w = np.asarray(gate_w, dtype=np.float32)

    # weight prep: [e, d] -> [p, c, e] with d = c*128 + p
    def wtr(m):
        return m.T.reshape(C, 128, E).transpose(1, 0, 2)

    whi = gate_w.astype(F16)
    wlo = ((gate_w - whi.astype(np.float32)) * np.float32(S_WLO)).astype(F16)
    w16 = np.ascontiguousarray(np.concatenate([wtr(whi), wtr(wlo)], axis=2))
    w8 = np.ascontiguousarray(wtr((gate_w * np.float32(S_W8)).astype(F8)))

    in_maps = []
    for i in range(B):
        xi = x[i]
        xhi = xi.astype(F16)
        xlo = ((xi - xhi.astype(np.float32)) * np.float32(S_XLO)).astype(F8)
        m = {"w16": w16, "w8": w8}
        for g, (t0, sz) in enumerate(GROUPS):
            m[f"xhi{g}"] = _permute(xhi[t0 : t0 + sz])
            m[f"xlo{g}"] = _permute(xlo[t0 : t0 + sz])
        in_maps.append(m)
    return in_maps


def kernel(x, gate_w):
    global _CACHED_NC
    if _CACHED_NC is None:
        _CACHED_NC = _build_nc()
    nc = _CACHED_NC

    in_maps = make_in_maps(x, gate_w)
    res = run_bass_kernel_spmd(nc, in_maps, list(range(B)))

    def unperm(a):  # [p, b, k] -> [t, k] with t = b*128 + p
        return a.transpose(1, 0, 2).reshape(T, -1)

    weights = np.stack([unperm(res.results[i]["w_out"]) for i in range(B)], axis=0)
    indices = np.stack(
        [unperm(res.results[i]["i_out"]) for i in range(B)], axis=0
    )
    return weights.astype(np.float32), indices.astype(np.int32)


# revision 15
# speedup vs baseline: 1.0669x; 1.0064x over previous
"""MoE router kernel (CityExpertRouter) for 8 Trainium2 NeuronCores.

reference:
    logits = einsum("bld,ed->ble", x[8,4096,2048]f32, gate_w[16,2048]f32)
    probs = softmax(logits); w, i = top_k(probs, 2); w /= w.sum(-1)
    returns (w [8,4096,2] f32, i [8,4096,2] i32)

Math simplification: softmax + top2 + renorm collapses to
    w1 = 1/(1+exp(l2-l1)), w2 = 1-w1   (l1, l2 = top-2 logits)
so only the top-2 logits (values + indices) are needed on-chip.

Strategy (DMA-bound problem; the cost floor is x bytes / DMA bandwidth):
  - Data parallel over batch: core i gets x[i] (4096 tokens).
  - 3-byte x encoding instead of 4: x = fp16(x) + 2^-11 * e3m4 residual.
    Host splits x into xhi = fp16(x) (2B) and xlo = e3m4((x-xhi)*2^11)
    (1B), cutting HBM traffic 25% below fp32 while keeping the logit
    quantization error ~2^-16 relative (top-2 index flips ~2/262144
    tokens, rel err ~2e-3, well under the 2e-2 gate).
  - Gate weight is consumed as two small moving operands:
      w16 = [fp16(w) | fp16((w-fp16(w))*2^11)]  (hi path, exact to 2^-23)
      w8  = e3m4(w*2^5)                          (lo path)
  - Token-major matmuls: the x chunk [128d, tokens] is the STATIONARY
    operand and the tiny gate weight [128d, 32|16] is the MOVING one, so
    each accumulation step costs only 32 (hi) / 16 (lo) PE cycles and
    the logits land directly as [tokens(partitions), expert-slots] in
    PSUM - no transpose/fold pass and no PSUM->SBUF logits copy at all.
    Per 128-token block: ps[:, 0:32] += xhi_c^T w16_c over 16 chunks,
    ps[:, 32:48] += xlo_c^T w8_c.
  - Per-block epilogue:
      * DVE scalar_tensor_tensor x2 straight from PSUM:
        l = (ps[:,32:48]*2^-16 + ps[:,0:16]) + ps[:,16:32]
      * DVE max/max_index (top-8 sorted) -> top-2 values+indices
      * ACT bias-AP sigmoids: w1 = sigmoid(-1*l2 + l1), w2 = sigmoid(-1*
        l1 + l2) straight from vals (no DVE sub needed)
  - Group sizes 15x256 then 128/64/32/32 (tail groups packed into the
    last staging block at partition offsets 64/0/32) so the pipeline
    trailing the final DMA byte is as short as possible.
  - Single final stores: w rides the scalar queue right behind its
    producer sigmoids, i takes the otherwise-idle SP queue in parallel;
    both transfer AFTER the x stream so they never contend with it.
  - Scheduling notes: const loads ride the scalar HWDGE queue so the SP
    queue is purely x-loads; only the top-2 indices are staged/stored.
"""

import numpy as np
import ml_dtypes

import concourse.bass as bass
import concourse.tile as tile
from concourse import bacc, mybir
from concourse.bass import ts
from concourse.bass_utils import run_bass_kernel_spmd

F16 = np.float16
F8 = ml_dtypes.float8_e3m4

B, L, D, E = 8, 4096, 2048, 16
T = L              # tokens per core (shard over batch dim)
C = D // 128       # 16 contraction chunks
NB = T // 128      # 32 staging blocks of 128 tokens

# groups: big steady-state groups, then a shrinking tail. (t0, size) pairs;
# tail groups are ordered so PSUM base partitions stay in {0,32,64} while
# the LAST group is small (short post-DMA pipeline).
GROUPS = [(i * 256, 256) for i in range(15)] + [
    (3840, 128),  # block 30
    (3968, 128),  # block 31 (the short tail group)
]
assert sum(sz for _, sz in GROUPS) == T

# power-of-two scales for the 3-way split (all exact in fp)
S_XLO = 2.0 ** 11   # x residual pre-scale
S_WLO = 2.0 ** 11   # w fp16 residual pre-scale
S_W8 = 2.0 ** 5     # w e3m4 pre-scale
S_LO = 1.0 / (S_XLO * S_W8)  # lo-psum fold scale 2^-16

_CACHED_NC = None


def _build_nc():
    dt = mybir.dt
    nc = bacc.Bacc(
        "TRN2", target_bir_lowering=False, debug=False, num_devices=B
    )
    xhi_d = [
        nc.dram_tensor(f"xhi{g}", [128, C, sz], dt.float16, kind="ExternalInput")
        for g, (_, sz) in enumerate(GROUPS)
    ]
    xlo_d = [
        nc.dram_tensor(f"xlo{g}", [128, C, sz], dt.float8e3, kind="ExternalInput")
        for g, (_, sz) in enumerate(GROUPS)
    ]
    w16_d = nc.dram_tensor("w16", [128, C, 2 * E], dt.float16, kind="ExternalInput")
    w8_d = nc.dram_tensor("w8", [128, C, E], dt.float8e3, kind="ExternalInput")
    # device-native layout [p, b, k]; host un-permutes to [token, k]
    wout_d = nc.dram_tensor("w_out", [128, NB, 2], dt.float32, kind="ExternalOutput")
    iout_d = nc.dram_tensor("i_out", [128, NB, 2], dt.uint32, kind="ExternalOutput")

    with tile.TileContext(nc) as tc:
        with (
            tc.tile_pool(name="consts", bufs=1) as consts,
            tc.tile_pool(name="xin", bufs=3) as xin,
            tc.tile_pool(name="work", bufs=4) as work,
            tc.tile_pool(name="psum", bufs=4, space="PSUM") as psum_pool,
        ):
            w16_sb = consts.tile([128, C, 2 * E], dt.float16)
            w8_sb = consts.tile([128, C, E], dt.float8e3)
            w_all = consts.tile([128, NB, 2], dt.float32)
            i_all = consts.tile([128, NB, 2], dt.uint32)

            for g, (t0, sz) in enumerate(GROUPS):
                xh = xin.tile([128, C, sz], dt.float16, name=f"xh_{sz}_{g % 3}")
                nc.sync.dma_start(xh[:], xhi_d[g][:])
                xl = xin.tile([128, C, sz], dt.float8e3, name=f"xl_{sz}_{g % 3}")
                nc.sync.dma_start(xl[:], xlo_d[g][:])
                if g == 0:
                    # consts go on the scalar HWDGE queue; SP queue stays
                    # pure x-loads
                    nc.scalar.dma_start(w16_sb[:], w16_d[:])
                    nc.scalar.dma_start(w8_sb[:], w8_d[:])

                nblk = max(1, sz // 128)
                po = t0 % 128  # 0 for full blocks; 0/32/64 for tail groups
                b0 = t0 // 128
                pn = min(sz, 128)

                # token-major accumulation, x stationary / w moving:
                #   ps[:, 0:16]=xhi@whi, [16:32]=xhi@wlo, [32:48]=xlo@w8
                pss = [
                    psum_pool.tile([128, 3 * E], dt.float32, name="ps")
                    for _ in range(nblk)
                ]
                # hi chains first (xhi arrives before xlo)
                for b, ps in enumerate(pss):
                    xs = xh[:, :, ts(b, 128)] if sz >= 128 else xh[:, :, :]
                    for c in range(C):
                        nc.tensor.matmul(
                            ps[po : po + pn, 0 : 2 * E],
                            xs[:, c, :],
                            w16_sb[:, c, :],
                            start=(c == 0),
                            stop=(c == C - 1),
                        )
                # pre-combine the hi psum slots on DVE while the lo bytes
                # are still in flight: hsum = whi-part + 2^-11 * wlo-part.
                # (HW allows only ONE PSUM input per DVE op, so two steps.)
                hsums = []
                for b, ps in enumerate(pss):
                    h1 = work.tile([128, E], dt.float32, name="h1")
                    nc.vector.tensor_scalar_mul(
                        h1[po : po + pn, :],
                        ps[po : po + pn, E : 2 * E],
                        1.0 / S_WLO,
                    )
                    hsum = work.tile([128, E], dt.float32, name="hsum")
                    nc.vector.scalar_tensor_tensor(
                        hsum[po : po + pn, :],
                        ps[po : po + pn, 0:E],
                        1.0,
                        h1[po : po + pn, :],
                        op0=mybir.AluOpType.mult,
                        op1=mybir.AluOpType.add,
                    )
                    hsums.append(hsum)
                    if g == len(GROUPS) - 1:
                        # last block: weights come from the UNcorrected hi
                        # logits (ready before the final lo bytes land) so
                        # the w-store never sits on the critical path. The
                        # residual only matters at near-ties, where both
                        # weights are ~0.5 either way; indices still use the
                        # corrected logits below.
                        vals_u = work.tile([128, 8], dt.float32, name="vals_u")
                        nc.vector.max(vals_u[po : po + pn, :], hsum[po : po + pn, :])
                        blk = b0 + b
                        nc.scalar.activation(
                            w_all[po : po + pn, blk, 0:1],
                            vals_u[po : po + pn, 1:2],
                            mybir.ActivationFunctionType.Sigmoid,
                            bias=vals_u[po : po + pn, 0:1],
                            scale=-1.0,
                        )
                        nc.scalar.activation(
                            w_all[po : po + pn, blk, 1:2],
                            vals_u[po : po + pn, 0:1],
                            mybir.ActivationFunctionType.Sigmoid,
                            bias=vals_u[po : po + pn, 1:2],
                            scale=-1.0,
                        )
                        nc.scalar.dma_start(
                            wout_d[:, NB - 1 :], w_all[:, NB - 1 :]
                        )

                for b, ps in enumerate(pss):
                    xs = xl[:, :, ts(b, 128)] if sz >= 128 else xl[:, :, :]
                    for c in range(C):
                        nc.tensor.matmul(
                            ps[po : po + pn, 2 * E : 3 * E],
                            xs[:, c, :],
                            w8_sb[:, c, :],
                            start=(c == 0),
                            stop=(c == C - 1),
                        )

                for b, ps in enumerate(pss):
                    blk = b0 + b
                    # single post-lo DVE op: l = 2^-16 * lo-part + hsum
                    lg = work.tile([128, E], dt.float32, name="lg")
                    nc.vector.scalar_tensor_tensor(
                        lg[po : po + pn, :],
                        ps[po : po + pn, 2 * E : 3 * E],
                        S_LO,
                        hsums[b][po : po + pn, :],
                        op0=mybir.AluOpType.mult,
                        op1=mybir.AluOpType.add,
                    )

                    vals = work.tile([128, 8], dt.float32, name="vals")
                    idx8 = work.tile([128, 8], dt.uint32, name="idx8")
                    nc.vector.max(vals[po : po + pn, :], lg[po : po + pn, :])
                    nc.vector.max_index(
                        idx8[po : po + pn, :],
                        vals[po : po + pn, :],
                        lg[po : po + pn, :],
                    )
                    if g < len(GROUPS) - 1:
                        # stage the top-2 indices (uint32 -> int32 on host)
                        nc.vector.tensor_copy(
                            i_all[po : po + pn, blk, :], idx8[po : po + pn, 0:2]
                        )
                        # w1 = sigmoid(l1-l2), w2 = sigmoid(l2-l1): bias-AP
                        nc.scalar.activation(
                            w_all[po : po + pn, blk, 0:1],
                            vals[po : po + pn, 1:2],
                            mybir.ActivationFunctionType.Sigmoid,
                            bias=vals[po : po + pn, 0:1],
                            scale=-1.0,
                        )
                        nc.scalar.activation(
                            w_all[po : po + pn, blk, 1:2],
                            vals[po : po + pn, 0:1],
                            mybir.ActivationFunctionType.Sigmoid,
                            bias=vals[po : po + pn, 1:2],
                            scale=-1.0,
                        )
                    else:
                        # last block: skip staging (weights were already
                        # produced from the uncorrected logits above); the
                        # tail i-store reads straight from the top-8 scratch
                        idx_last = idx8

            # final stores, split so the only piece gated by the last block
            # is the tiny i slice on the otherwise-idle SP queue
            nc.sync.dma_start(iout_d[:, : NB - 1], i_all[:, : NB - 1])
            nc.scalar.dma_start(wout_d[:, : NB - 1], w_all[:, : NB - 1])
            nc.sync.dma_start(iout_d[:, NB - 1 :], idx_last[:, 0:2])

    nc.compile()
    return nc


def _permute(m):
    """[sz, D] -> [p=128, c, sz] device layout (d = c*128 + p)."""
    sz = m.shape[0]
    return np.ascontiguousarray(m.reshape(sz, C, 128).transpose(2, 1, 0))


def make_in_maps(x, gate_w):
    x = np.asarray(x, dtype=np.float32)
    gate_w = np.asarray(gate_w, dtype=np.float32)

    # weight prep: [e, d] -> [p, c, e] with d = c*128 + p
    def wtr(m):
        return m.T.reshape(C, 128, E).transpose(1, 0, 2)

    whi = gate_w.astype(F16)
    wlo = ((gate_w - whi.astype(np.float32)) * np.float32(S_WLO)).astype(F16)
    w16 = np.ascontiguousarray(np.concatenate([wtr(whi), wtr(wlo)], axis=2))
    w8 = np.ascontiguousarray(wtr((gate_w * np.float32(S_W8)).astype(F8)))

    in_maps = []
    for i in range(B):
        xi = x[i]
        xhi = xi.astype(F16)
        xlo = ((xi - xhi.astype(np.float32)) * np.float32(S_XLO)).astype(F8)
        m = {"w16": w16, "w8": w8}
        for g, (t0, sz) in enumerate(GROUPS):
            m[f"xhi{g}"] = _permute(xhi[t0 : t0 + sz])
            m[f"xlo{g}"] = _permute(xlo[t0 : t0 + sz])
        in_maps.append(m)
    return in_maps


def kernel(x, gate_w):
    global _CACHED_NC
    if _CACHED_NC is None:
        _CACHED_NC = _build_nc()
    nc = _CACHED_NC

    in_maps = make_in_maps(x, gate_w)
    res = run_bass_kernel_spmd(nc, in_maps, list(range(B)))

    def unperm(a):  # [p, b, k] -> [t, k] with t = b*128 + p
        return a.transpose(1, 0, 2).reshape(T, -1)

    weights = np.stack([unperm(res.results[i]["w_out"]) for i in range(B)], axis=0)
    indices = np.stack(
        [unperm(res.results[i]["i_out"]) for i in range(B)], axis=0
    )
    return weights.astype(np.float32), indices.astype(np.int32)
